# revision 19
# baseline (speedup 1.0000x reference)
"""Trainium2 Bass kernel for the differentiable gaussian renderer.

Math: for image b with camera origin o and (unnormalized) ray dirs d_p,
  t      = (m - o) . d
  pdiff  = (m - o) - d t
  dist2  = |pdiff|^2 = diff^T (I - (2-|d|^2) d d^T) diff        (quadratic form)
  w[p,n] = exp(-0.5 dist2 / (mean(diag(cov_n)) + eps))
  rgb    = (w @ colors) / (w.sum(-1) + eps)

So with q_n = -0.5/sigma'_n * [dx^2,dy^2,dz^2,2dxdy,2dxdz,2dydz]  (per gaussian)
and A6_p = sym6(I - c_p d_p d_p^T)                               (per ray)
the exp argument is a K=6 matmul  arg[n,p] = q_n . A6_p  (always <= 0).

Device pipeline per 128-gaussian chunk (data-parallel: core = (image, 1024-ray
block), gaussians replicated):
  PE  : arg = Q[g]^T A6        (K=18: bf16 hi/lo split for ~f32 accuracy)
  ACT : w = exp(arg)           (PSUM -> SBUF, fp16)
  PE  : acc[4,rays] += colors4[g]^T w   (PSUM accumulate, 128 chunks)
Host: divide num/(den+eps) and assemble.
"""

import sys

if "/opt/trn_rl_repo" not in sys.path:
    sys.path.insert(0, "/opt/trn_rl_repo")

from contextlib import ExitStack

import ml_dtypes
import numpy as np

TILE = 64
P_IMG = TILE * TILE          # 4096 rays per image
N_GAUSS = 16384
N_IMGS = 2
N_CORES = 8
P_CORE = P_IMG * N_IMGS // N_CORES   # 1024 rays per core
EPS = 1e-5
NCHUNK = N_GAUSS // 128      # 128 gaussian chunks
KSPLIT = 96                  # 18 used (6*(qh.Ah+ql.Ah+qh.Al)), zero-padded to 96
                             # (small-K matmuls read as "idle" to the PE clock-gate HAM
                             #  and the PE never leaves 1.2GHz; measured: K<=64 stays
                             #  cold, K>=96 reaches 2.4GHz)

_module_cache = {}


def _split_f16(x):
    """x (float64) -> (hi, lo) fp16 with hi+lo ~= x to ~2^-21 rel."""
    hi = x.astype(np.float16)
    lo = (x - hi.astype(np.float64)).astype(np.float16)
    return hi, lo


def _host_prep(means, covs, colors, intrinsics, extrinsics, image_ids):
    """Build per-core device inputs."""
    means = np.asarray(means, np.float64)
    covs = np.asarray(covs, np.float64)
    colors = np.asarray(colors, np.float64)
    intrinsics = np.asarray(intrinsics, np.float64)
    extrinsics = np.asarray(extrinsics, np.float64)
    image_ids = np.asarray(image_ids).astype(np.int64)

    sig = covs[:, [0, 1, 2], [0, 1, 2]].mean(-1) + EPS        # [N] sigma^2+eps
    s = -0.5 / sig                                            # [N]

    # colors4 = [r,g,b,1] tiled for lhsT chunks: [128, 4*NCHUNK] fp16
    colors4 = np.concatenate([colors, np.ones((N_GAUSS, 1))], -1)  # [N,4]
    col_tiled = (
        colors4.reshape(NCHUNK, 128, 4).transpose(1, 0, 2).reshape(128, 4 * NCHUNK)
    ).astype(np.float16)

    qs, ds = [], []
    for b in range(N_IMGS):
        cam = int(image_ids[b])
        fx, fy, cx, cy = intrinsics[cam]
        c2w = extrinsics[cam]
        i, j = np.meshgrid(np.arange(TILE), np.arange(TILE), indexing="xy")
        dirs = np.stack(
            [(i - cx) / fx, -(j - cy) / fy, -np.ones_like(i, np.float64)], -1
        ).reshape(-1, 3)                                      # [P,3]
        rays_d = dirs @ c2w[:3, :3].T                         # [P,3]
        o = c2w[:3, 3]

        c = 2.0 - (rays_d**2).sum(-1)                         # [P]
        A = np.eye(3)[None] - c[:, None, None] * rays_d[:, :, None] * rays_d[:, None, :]
        A6 = np.stack(
            [A[:, 0, 0], A[:, 1, 1], A[:, 2, 2], A[:, 0, 1], A[:, 0, 2], A[:, 1, 2]], 0
        )                                                     # [6,P]

        diff = means - o                                      # [N,3]
        dx, dy, dz = diff[:, 0], diff[:, 1], diff[:, 2]
        q6 = np.stack([dx * dx, dy * dy, dz * dz, 2 * dx * dy, 2 * dx * dz, 2 * dy * dz], 0)
        Q = s[None, :] * q6                                   # [6,N]

        qh, ql = _split_f16(Q)
        ah, al = _split_f16(A6)
        qs.append(np.concatenate([qh, ql, qh], 0))            # [18,N] fp16
        ds.append(np.concatenate([ah, ah, al], 0))            # [18,P] fp16

    in_maps = []
    for core in range(N_CORES):
        b = core // (N_CORES // N_IMGS)
        rb = core % (N_CORES // N_IMGS)
        in_maps.append(
            {
                "q": np.ascontiguousarray(qs[b]),
                "d": np.ascontiguousarray(ds[b][:, rb * P_CORE : (rb + 1) * P_CORE]),
                "col": col_tiled,
            }
        )
    return in_maps


def _build_module():
    if "nc" in _module_cache:
        return _module_cache["nc"]
    import concourse.bacc as bacc
    import concourse.mybir as mybir
    import concourse.tile as tile

    nc = bacc.Bacc("TRN2", target_bir_lowering=False, debug=False)
    q = nc.dram_tensor("q", [18, N_GAUSS], mybir.dt.float16, kind="ExternalInput")
    d = nc.dram_tensor("d", [18, P_CORE], mybir.dt.float16, kind="ExternalInput")
    col = nc.dram_tensor("col", [128, 4 * NCHUNK], mybir.dt.float16, kind="ExternalInput")
    out = nc.dram_tensor("out", [4, P_CORE], mybir.dt.float32, kind="ExternalOutput")

    f32 = mybir.dt.float32
    Exp = mybir.ActivationFunctionType.Exp

    # q pieces (uneven: tiny piece 0 so compute starts ~immediately; later
    # pieces' memset+DMA hide under compute)
    PIECES = [512, 3584, 6144, 6144]
    POFF = [0, 512, 4096, 10240]
    NSLOT = 2 * NCHUNK               # 256 [128,512]-col units of (chunk, rayblock)
    # ACT groups alternate 4 and 3 slots (7 PSUM banks, both accumulators
    # packed into the 8th via tile_position col groups)
    GSIZES = [2]                     # small first group -> first exp starts sooner
    while sum(GSIZES) < NSLOT:
        GSIZES.append(min(4 if len(GSIZES) % 2 == 0 else 3, NSLOT - sum(GSIZES)))
    GOFF = [sum(GSIZES[:i]) for i in range(len(GSIZES))]
    NGRP = len(GSIZES)

    with tile.TileContext(nc) as tc, ExitStack() as ctx:
        const = ctx.enter_context(tc.tile_pool(name="const", bufs=1))
        wpool = ctx.enter_context(tc.tile_pool(name="w", bufs=2))
        argp = ctx.enter_context(tc.tile_pool(name="arg", bufs=1, space="PSUM"))
        accp = ctx.enter_context(tc.tile_pool(name="acc", bufs=1, space="PSUM"))
        outp = ctx.enter_context(tc.tile_pool(name="outp", bufs=1))

        # warm the exp table while DMAs run
        dummy = const.tile([1, 16], f32, tag="dummy")
        nc.vector.memset(dummy[:], 0.0)
        nc.scalar.activation(dummy[:], dummy[:], Exp)

        # q pieces: DMA the 18 real K-rows, memset the zero pad. Critical
        # path = d_sb + piece 0 only; later pieces fill during compute.
        d_sb = const.tile([KSPLIT, P_CORE], mybir.dt.float16, tag="d")
        nc.vector.memset(d_sb[:], 0.0)
        nc.sync.dma_start(d_sb[0:18, :], d.ap())
        qts = []
        for p, (off, w_) in enumerate(zip(POFF, PIECES)):
            qt = const.tile([KSPLIT, w_], mybir.dt.float16, tag=f"q{p}")
            nc.gpsimd.memset(qt[:], 0.0)
            nc.sync.dma_start(qt[0:18, :], q.ap()[:, off : off + w_])
            qts.append(qt)
        col_sb = const.tile([128, 4 * NCHUNK], mybir.dt.float16, tag="col")
        nc.sync.dma_start(col_sb[:], col.ap())

        # both ray-block accumulators packed into ONE psum bank: acc0 at
        # partitions 0-3 (col group 0), acc1 at partitions 32-35 (col group
        # 1, via tile_position). Col-group MMs execute concurrently in the
        # array, so a start=True on one of them races the other's first
        # write; instead a zero matmul (K-pad zero rows x zero d rows)
        # claims the whole bank with start=True, and every color MM then
        # accumulates with start=False.
        acc_all = accp.tile([128, 512], f32, tag="acc")
        nc.tensor.matmul(acc_all[:], qts[0][32:64, 0:128], d_sb[32:64, 0:512],
                         start=True, stop=False, skip_group_check=True)

        def slots_of(h):
            return range(GOFF[h], GOFF[h] + GSIZES[h])

        SKEWG = 2
        whs = [None] * NGRP

        def color_mms(h):
            w = whs[h]
            for k, s in enumerate(slots_of(h)):
                g, rb = s // 2, s % 2
                clhs = col_sb[:, g * 4 : (g + 1) * 4]
                nc.tensor.matmul(acc_all[32 * rb : 32 * rb + 4, :], clhs,
                                 w[:, k * 512 : (k + 1) * 512],
                                 start=False, stop=(s >= NSLOT - 2),
                                 tile_position=(0, 32 * rb),
                                 skip_group_check=True)
            whs[h] = None

        for h in range(NGRP):
            ns = GSIZES[h]
            targ = argp.tile([128, ns * 512], f32, tag=f"arg{h % 2}")
            for k, s in enumerate(slots_of(h)):
                g, rb = s // 2, s % 2
                p = next(i for i in range(len(POFF)) if POFF[i] <= g * 128 < POFF[i] + PIECES[i])
                lhs = qts[p][:, g * 128 - POFF[p] :][:, 0:128]
                nc.tensor.matmul(targ[:, k * 512 : (k + 1) * 512], lhs,
                                 d_sb[:, rb * 512 : (rb + 1) * 512],
                                 start=True, stop=True)

            if h >= SKEWG:
                color_mms(h - SKEWG)

            w = wpool.tile([128, ns * 512], mybir.dt.float16, tag=f"w{h % 2}")
            nc.scalar.activation(w[:], targ[:], Exp)
            whs[h] = w

        for h in range(NGRP - SKEWG, NGRP):
            color_mms(h)

        out_sb = outp.tile([4, P_CORE], f32)
        nc.vector.tensor_copy(out_sb[:, 0:512], acc_all[0:4, :])
        nc.vector.tensor_copy(out_sb[:, 512:1024], acc_all[32:36, :])
        nc.sync.dma_start(out.ap(), out_sb[:])

    nc.compile()
    _module_cache["nc"] = nc
    return nc


def _run(in_maps, trace=False, **kw):
    from concourse import bass_utils

    nc = _build_module()
    return bass_utils.run_bass_kernel_spmd(
        nc, in_maps, core_ids=list(range(N_CORES)), trace=trace, **kw
    )


def _assemble(results):
    out = np.zeros((N_IMGS, 3, P_IMG), np.float32)
    for core in range(N_CORES):
        b = core // (N_CORES // N_IMGS)
        rb = core % (N_CORES // N_IMGS)
        o = results[core]["out"]                    # [4, P_CORE] f32
        rgb = o[:3] / (o[3:4] + EPS)
        out[b, :, rb * P_CORE : (rb + 1) * P_CORE] = rgb
    return out


def kernel(means, covs, colors, intrinsics, extrinsics, image_ids):
    in_maps = _host_prep(means, covs, colors, intrinsics, extrinsics, image_ids)
    res = _run(in_maps)
    return _assemble(res.results)


# revision 20
# speedup vs baseline: 1.0050x; 1.0050x over previous
"""Trainium2 Bass kernel for the differentiable gaussian renderer.

Math: for image b with camera origin o and (unnormalized) ray dirs d_p,
  t      = (m - o) . d
  pdiff  = (m - o) - d t
  dist2  = |pdiff|^2 = diff^T (I - (2-|d|^2) d d^T) diff        (quadratic form)
  w[p,n] = exp(-0.5 dist2 / (mean(diag(cov_n)) + eps))
  rgb    = (w @ colors) / (w.sum(-1) + eps)

So with q_n = -0.5/sigma'_n * [dx^2,dy^2,dz^2,2dxdy,2dxdz,2dydz]  (per gaussian)
and A6_p = sym6(I - c_p d_p d_p^T)                               (per ray)
the exp argument is a K=6 matmul  arg[n,p] = q_n . A6_p  (always <= 0).

Device pipeline per 128-gaussian chunk (data-parallel: core = (image, 1024-ray
block), gaussians replicated):
  PE  : arg = Q[g]^T A6        (K=18: bf16 hi/lo split for ~f32 accuracy)
  ACT : w = exp(arg)           (PSUM -> SBUF, fp16)
  PE  : acc[4,rays] += colors4[g]^T w   (PSUM accumulate, 128 chunks)
Host: divide num/(den+eps) and assemble.
"""

import sys

if "/opt/trn_rl_repo" not in sys.path:
    sys.path.insert(0, "/opt/trn_rl_repo")

from contextlib import ExitStack

import ml_dtypes
import numpy as np

TILE = 64
P_IMG = TILE * TILE          # 4096 rays per image
N_GAUSS = 16384
N_IMGS = 2
N_CORES = 8
P_CORE = P_IMG * N_IMGS // N_CORES   # 1024 rays per core
EPS = 1e-5
NCHUNK = N_GAUSS // 128      # 128 gaussian chunks
KSPLIT = 96                  # 18 used (6*(qh.Ah+ql.Ah+qh.Al)), zero-padded to 96
                             # (small-K matmuls read as "idle" to the PE clock-gate HAM
                             #  and the PE never leaves 1.2GHz; measured: K<=64 stays
                             #  cold, K>=96 reaches 2.4GHz)

_module_cache = {}


def _split_f16(x):
    """x (float64) -> (hi, lo) fp16 with hi+lo ~= x to ~2^-21 rel."""
    hi = x.astype(np.float16)
    lo = (x - hi.astype(np.float64)).astype(np.float16)
    return hi, lo


def _host_prep(means, covs, colors, intrinsics, extrinsics, image_ids):
    """Build per-core device inputs."""
    means = np.asarray(means, np.float64)
    covs = np.asarray(covs, np.float64)
    colors = np.asarray(colors, np.float64)
    intrinsics = np.asarray(intrinsics, np.float64)
    extrinsics = np.asarray(extrinsics, np.float64)
    image_ids = np.asarray(image_ids).astype(np.int64)

    sig = covs[:, [0, 1, 2], [0, 1, 2]].mean(-1) + EPS        # [N] sigma^2+eps
    s = -0.5 / sig                                            # [N]

    # colors4 = [r,g,b,1] tiled for lhsT chunks: [128, 4*NCHUNK] fp16
    colors4 = np.concatenate([colors, np.ones((N_GAUSS, 1))], -1)  # [N,4]
    col_tiled = (
        colors4.reshape(NCHUNK, 128, 4).transpose(1, 0, 2).reshape(128, 4 * NCHUNK)
    ).astype(np.float16)

    qs, ds = [], []
    for b in range(N_IMGS):
        cam = int(image_ids[b])
        fx, fy, cx, cy = intrinsics[cam]
        c2w = extrinsics[cam]
        i, j = np.meshgrid(np.arange(TILE), np.arange(TILE), indexing="xy")
        dirs = np.stack(
            [(i - cx) / fx, -(j - cy) / fy, -np.ones_like(i, np.float64)], -1
        ).reshape(-1, 3)                                      # [P,3]
        rays_d = dirs @ c2w[:3, :3].T                         # [P,3]
        o = c2w[:3, 3]

        c = 2.0 - (rays_d**2).sum(-1)                         # [P]
        A = np.eye(3)[None] - c[:, None, None] * rays_d[:, :, None] * rays_d[:, None, :]
        A6 = np.stack(
            [A[:, 0, 0], A[:, 1, 1], A[:, 2, 2], A[:, 0, 1], A[:, 0, 2], A[:, 1, 2]], 0
        )                                                     # [6,P]

        diff = means - o                                      # [N,3]
        dx, dy, dz = diff[:, 0], diff[:, 1], diff[:, 2]
        q6 = np.stack([dx * dx, dy * dy, dz * dz, 2 * dx * dy, 2 * dx * dz, 2 * dy * dz], 0)
        Q = s[None, :] * q6                                   # [6,N]

        qh, ql = _split_f16(Q)
        ah, al = _split_f16(A6)
        qs.append(np.concatenate([qh, ql, qh], 0))            # [18,N] fp16
        ds.append(np.concatenate([ah, ah, al], 0))            # [18,P] fp16

    in_maps = []
    for core in range(N_CORES):
        b = core // (N_CORES // N_IMGS)
        rb = core % (N_CORES // N_IMGS)
        in_maps.append(
            {
                "q": np.ascontiguousarray(qs[b]),
                "d": np.ascontiguousarray(ds[b][:, rb * P_CORE : (rb + 1) * P_CORE]),
                "col": col_tiled,
            }
        )
    return in_maps


def _build_module():
    if "nc" in _module_cache:
        return _module_cache["nc"]
    import concourse.bacc as bacc
    import concourse.mybir as mybir
    import concourse.tile as tile

    nc = bacc.Bacc("TRN2", target_bir_lowering=False, debug=False)
    q = nc.dram_tensor("q", [18, N_GAUSS], mybir.dt.float16, kind="ExternalInput")
    d = nc.dram_tensor("d", [18, P_CORE], mybir.dt.float16, kind="ExternalInput")
    col = nc.dram_tensor("col", [128, 4 * NCHUNK], mybir.dt.float16, kind="ExternalInput")
    out = nc.dram_tensor("out", [4, P_CORE], mybir.dt.float32, kind="ExternalOutput")

    f32 = mybir.dt.float32
    Exp = mybir.ActivationFunctionType.Exp

    # q pieces (uneven: tiny piece 0 so compute starts ~immediately; later
    # pieces' memset+DMA hide under compute)
    PIECES = [512, 1536, 2048, 6144, 6144]
    POFF = [0, 512, 2048, 4096, 10240]
    NSLOT = 2 * NCHUNK               # 256 [128,512]-col units of (chunk, rayblock)
    # ACT groups alternate 4 and 3 slots (7 PSUM banks, both accumulators
    # packed into the 8th via tile_position col groups)
    GSIZES = [2]                     # small first group -> first exp starts sooner
    while sum(GSIZES) < NSLOT:
        GSIZES.append(min(4 if len(GSIZES) % 2 == 0 else 3, NSLOT - sum(GSIZES)))
    GOFF = [sum(GSIZES[:i]) for i in range(len(GSIZES))]
    NGRP = len(GSIZES)

    with tile.TileContext(nc) as tc, ExitStack() as ctx:
        const = ctx.enter_context(tc.tile_pool(name="const", bufs=1))
        wpool = ctx.enter_context(tc.tile_pool(name="w", bufs=2))
        argp = ctx.enter_context(tc.tile_pool(name="arg", bufs=1, space="PSUM"))
        accp = ctx.enter_context(tc.tile_pool(name="acc", bufs=1, space="PSUM"))
        outp = ctx.enter_context(tc.tile_pool(name="outp", bufs=1))

        # warm the exp table while DMAs run
        dummy = const.tile([1, 16], f32, tag="dummy")
        nc.vector.memset(dummy[:], 0.0)
        nc.scalar.activation(dummy[:], dummy[:], Exp)

        # q pieces: DMA the 18 real K-rows, memset the zero pad. Critical
        # path = d_sb + piece 0 only; later pieces fill during compute.
        d_sb = const.tile([KSPLIT, P_CORE], mybir.dt.float16, tag="d")
        nc.vector.memset(d_sb[:], 0.0)
        nc.sync.dma_start(d_sb[0:18, :], d.ap())
        qts = []
        for p, (off, w_) in enumerate(zip(POFF, PIECES)):
            qt = const.tile([KSPLIT, w_], mybir.dt.float16, tag=f"q{p}")
            nc.gpsimd.memset(qt[:], 0.0)
            nc.sync.dma_start(qt[0:18, :], q.ap()[:, off : off + w_])
            qts.append(qt)
        col_sb = const.tile([128, 4 * NCHUNK], mybir.dt.float16, tag="col")
        nc.sync.dma_start(col_sb[:], col.ap())

        # both ray-block accumulators packed into ONE psum bank: acc0 at
        # partitions 0-3 (col group 0), acc1 at partitions 32-35 (col group
        # 1, via tile_position). Col-group MMs execute concurrently in the
        # array, so a start=True on one of them races the other's first
        # write; instead a zero matmul (K-pad zero rows x zero d rows)
        # claims the whole bank with start=True, and every color MM then
        # accumulates with start=False.
        acc_all = accp.tile([128, 512], f32, tag="acc")
        nc.tensor.matmul(acc_all[:], qts[0][32:64, 0:128], d_sb[32:64, 0:512],
                         start=True, stop=False, skip_group_check=True)

        def slots_of(h):
            return range(GOFF[h], GOFF[h] + GSIZES[h])

        SKEWG = 2
        whs = [None] * NGRP

        def color_mms(h):
            w = whs[h]
            for k, s in enumerate(slots_of(h)):
                g, rb = s // 2, s % 2
                clhs = col_sb[:, g * 4 : (g + 1) * 4]
                nc.tensor.matmul(acc_all[32 * rb : 32 * rb + 4, :], clhs,
                                 w[:, k * 512 : (k + 1) * 512],
                                 start=False, stop=(s >= NSLOT - 2),
                                 tile_position=(0, 32 * rb),
                                 skip_group_check=True)
            whs[h] = None

        for h in range(NGRP):
            ns = GSIZES[h]
            targ = argp.tile([128, ns * 512], f32, tag=f"arg{h % 2}")
            for k, s in enumerate(slots_of(h)):
                g, rb = s // 2, s % 2
                p = next(i for i in range(len(POFF)) if POFF[i] <= g * 128 < POFF[i] + PIECES[i])
                lhs = qts[p][:, g * 128 - POFF[p] :][:, 0:128]
                nc.tensor.matmul(targ[:, k * 512 : (k + 1) * 512], lhs,
                                 d_sb[:, rb * 512 : (rb + 1) * 512],
                                 start=True, stop=True)

            if h >= SKEWG:
                color_mms(h - SKEWG)

            w = wpool.tile([128, ns * 512], mybir.dt.float16, tag=f"w{h % 2}")
            nc.scalar.activation(w[:], targ[:], Exp)
            whs[h] = w

        for h in range(NGRP - SKEWG, NGRP):
            color_mms(h)

        out_sb = outp.tile([4, P_CORE], f32)
        nc.vector.tensor_copy(out_sb[:, 0:512], acc_all[0:4, :])
        nc.vector.tensor_copy(out_sb[:, 512:1024], acc_all[32:36, :])
        nc.sync.dma_start(out.ap(), out_sb[:])

    nc.compile()
    _module_cache["nc"] = nc
    return nc


def _run(in_maps, trace=False, **kw):
    from concourse import bass_utils

    nc = _build_module()
    return bass_utils.run_bass_kernel_spmd(
        nc, in_maps, core_ids=list(range(N_CORES)), trace=trace, **kw
    )


def _assemble(results):
    out = np.zeros((N_IMGS, 3, P_IMG), np.float32)
    for core in range(N_CORES):
        b = core // (N_CORES // N_IMGS)
        rb = core % (N_CORES // N_IMGS)
        o = results[core]["out"]                    # [4, P_CORE] f32
        rgb = o[:3] / (o[3:4] + EPS)
        out[b, :, rb * P_CORE : (rb + 1) * P_CORE] = rgb
    return out


def kernel(means, covs, colors, intrinsics, extrinsics, image_ids):
    in_maps = _host_prep(means, covs, colors, intrinsics, extrinsics, image_ids)
    res = _run(in_maps)
    return _assemble(res.results)


# revision 21
# speedup vs baseline: 1.0140x; 1.0090x over previous
"""Trainium2 Bass kernel for the differentiable gaussian renderer.

Math: for image b with camera origin o and (unnormalized) ray dirs d_p,
  t      = (m - o) . d
  pdiff  = (m - o) - d t
  dist2  = |pdiff|^2 = diff^T (I - (2-|d|^2) d d^T) diff        (quadratic form)
  w[p,n] = exp(-0.5 dist2 / (mean(diag(cov_n)) + eps))
  rgb    = (w @ colors) / (w.sum(-1) + eps)

So with q_n = -0.5/sigma'_n * [dx^2,dy^2,dz^2,2dxdy,2dxdz,2dydz]  (per gaussian)
and A6_p = sym6(I - c_p d_p d_p^T)                               (per ray)
the exp argument is a K=6 matmul  arg[n,p] = q_n . A6_p  (always <= 0).

Device pipeline per 128-gaussian chunk (data-parallel: core = (image, 1024-ray
block), gaussians replicated):
  PE  : arg = Q[g]^T A6   (18 fp16 rows: hi/lo split of q and A6 for ~f32
                           accuracy; zero-padded to K=96 because small-K
                           matmuls look idle to the PE clock-gate and the
                           PE then never leaves its cold 1.2 GHz clock)
  ACT : w = exp(arg)      (PSUM -> SBUF, fp16, ops grouped [128,2048]/[128,1536])
  PE  : acc[4,rays] += colors4[g]^T w   (PSUM accumulate over all 128 chunks,
                           both ray-block accumulators packed in one bank)
Host: divide num/(den+eps) and assemble.
"""

import sys

if "/opt/trn_rl_repo" not in sys.path:
    sys.path.insert(0, "/opt/trn_rl_repo")

from contextlib import ExitStack

import numpy as np

TILE = 64
P_IMG = TILE * TILE          # 4096 rays per image
N_GAUSS = 16384
N_IMGS = 2
N_CORES = 8
P_CORE = P_IMG * N_IMGS // N_CORES   # 1024 rays per core
EPS = 1e-5
NCHUNK = N_GAUSS // 128      # 128 gaussian chunks
KSPLIT = 96                  # 18 used (6*(qh.Ah+ql.Ah+qh.Al)), zero-padded to 96
                             # (small-K matmuls read as "idle" to the PE clock-gate HAM
                             #  and the PE never leaves 1.2GHz; measured: K<=64 stays
                             #  cold, K>=96 reaches 2.4GHz)

_module_cache = {}


def _split_f16(x):
    """x (float64) -> (hi, lo) fp16 with hi+lo ~= x to ~2^-21 rel."""
    hi = x.astype(np.float16)
    lo = (x - hi.astype(np.float64)).astype(np.float16)
    return hi, lo


def _host_prep(means, covs, colors, intrinsics, extrinsics, image_ids):
    """Build per-core device inputs."""
    means = np.asarray(means, np.float64)
    covs = np.asarray(covs, np.float64)
    colors = np.asarray(colors, np.float64)
    intrinsics = np.asarray(intrinsics, np.float64)
    extrinsics = np.asarray(extrinsics, np.float64)
    image_ids = np.asarray(image_ids).astype(np.int64)

    sig = covs[:, [0, 1, 2], [0, 1, 2]].mean(-1) + EPS        # [N] sigma^2+eps
    s = -0.5 / sig                                            # [N]

    # colors4 = [r,g,b,1] tiled for lhsT chunks: [128, 4*NCHUNK] fp16
    colors4 = np.concatenate([colors, np.ones((N_GAUSS, 1))], -1)  # [N,4]
    col_tiled = (
        colors4.reshape(NCHUNK, 128, 4).transpose(1, 0, 2).reshape(128, 4 * NCHUNK)
    ).astype(np.float16)

    qs, ds = [], []
    for b in range(N_IMGS):
        cam = int(image_ids[b])
        fx, fy, cx, cy = intrinsics[cam]
        c2w = extrinsics[cam]
        i, j = np.meshgrid(np.arange(TILE), np.arange(TILE), indexing="xy")
        dirs = np.stack(
            [(i - cx) / fx, -(j - cy) / fy, -np.ones_like(i, np.float64)], -1
        ).reshape(-1, 3)                                      # [P,3]
        rays_d = dirs @ c2w[:3, :3].T                         # [P,3]
        o = c2w[:3, 3]

        c = 2.0 - (rays_d**2).sum(-1)                         # [P]
        A = np.eye(3)[None] - c[:, None, None] * rays_d[:, :, None] * rays_d[:, None, :]
        A6 = np.stack(
            [A[:, 0, 0], A[:, 1, 1], A[:, 2, 2], A[:, 0, 1], A[:, 0, 2], A[:, 1, 2]], 0
        )                                                     # [6,P]

        diff = means - o                                      # [N,3]
        dx, dy, dz = diff[:, 0], diff[:, 1], diff[:, 2]
        q6 = np.stack([dx * dx, dy * dy, dz * dz, 2 * dx * dy, 2 * dx * dz, 2 * dy * dz], 0)
        Q = s[None, :] * q6                                   # [6,N]

        qh, ql = _split_f16(Q)
        ah, al = _split_f16(A6)
        qs.append(np.concatenate([qh, ql, qh], 0))            # [18,N] fp16
        ds.append(np.concatenate([ah, ah, al], 0))            # [18,P] fp16

    in_maps = []
    for core in range(N_CORES):
        b = core // (N_CORES // N_IMGS)
        rb = core % (N_CORES // N_IMGS)
        in_maps.append(
            {
                "q": np.ascontiguousarray(qs[b]),
                "d": np.ascontiguousarray(ds[b][:, rb * P_CORE : (rb + 1) * P_CORE]),
                "col": col_tiled,
            }
        )
    return in_maps


def _build_module():
    if "nc" in _module_cache:
        return _module_cache["nc"]
    import concourse.bacc as bacc
    import concourse.mybir as mybir
    import concourse.tile as tile

    nc = bacc.Bacc("TRN2", target_bir_lowering=False, debug=False)
    q = nc.dram_tensor("q", [18, N_GAUSS], mybir.dt.float16, kind="ExternalInput")
    d = nc.dram_tensor("d", [18, P_CORE], mybir.dt.float16, kind="ExternalInput")
    col = nc.dram_tensor("col", [128, 4 * NCHUNK], mybir.dt.float16, kind="ExternalInput")
    out = nc.dram_tensor("out", [4, P_CORE], mybir.dt.float32, kind="ExternalOutput")

    f32 = mybir.dt.float32
    Exp = mybir.ActivationFunctionType.Exp

    # q pieces (uneven: tiny piece 0 so compute starts ~immediately; later
    # pieces' memset+DMA hide under compute)
    PIECES = [512, 1536, 2048, 6144, 6144]
    POFF = [0, 512, 2048, 4096, 10240]
    NSLOT = 2 * NCHUNK               # 256 [128,512]-col units of (chunk, rayblock)
    # ACT groups alternate 4 and 3 slots (7 PSUM banks, both accumulators
    # packed into the 8th via tile_position col groups)
    GSIZES = [2]                     # small first group -> first exp starts sooner
    while sum(GSIZES) < NSLOT:
        GSIZES.append(min(4 if len(GSIZES) % 2 == 0 else 3, NSLOT - sum(GSIZES)))
    GOFF = [sum(GSIZES[:i]) for i in range(len(GSIZES))]
    NGRP = len(GSIZES)

    with tile.TileContext(nc) as tc, ExitStack() as ctx:
        const = ctx.enter_context(tc.tile_pool(name="const", bufs=1))
        wpool = ctx.enter_context(tc.tile_pool(name="w", bufs=2))
        argp = ctx.enter_context(tc.tile_pool(name="arg", bufs=1, space="PSUM"))
        accp = ctx.enter_context(tc.tile_pool(name="acc", bufs=1, space="PSUM"))
        outp = ctx.enter_context(tc.tile_pool(name="outp", bufs=1))

        # warm the exp table while DMAs run
        dummy = const.tile([1, 16], f32, tag="dummy")
        nc.vector.memset(dummy[:], 0.0)
        nc.scalar.activation(dummy[:], dummy[:], Exp)

        # q pieces: DMA the 18 real K-rows, memset the zero pad. Critical
        # path = d_sb + piece 0 only; later pieces fill during compute.
        d_sb = const.tile([KSPLIT, P_CORE], mybir.dt.float16, tag="d")
        nc.vector.memset(d_sb[:], 0.0)
        nc.sync.dma_start(d_sb[0:18, :], d.ap())
        qts = []
        for p, (off, w_) in enumerate(zip(POFF, PIECES)):
            qt = const.tile([KSPLIT, w_], mybir.dt.float16, tag=f"q{p}")
            nc.gpsimd.memset(qt[:], 0.0)
            nc.sync.dma_start(qt[0:18, :], q.ap()[:, off : off + w_])
            qts.append(qt)
        col_sb = const.tile([128, 4 * NCHUNK], mybir.dt.float16, tag="col")
        nc.sync.dma_start(col_sb[:], col.ap())

        # both ray-block accumulators packed into ONE psum bank: acc0 at
        # partitions 0-3 (col group 0), acc1 at partitions 32-35 (col group
        # 1, via tile_position). Col-group MMs execute concurrently in the
        # array, so a start=True on one of them races the other's first
        # write; instead a zero matmul (K-pad zero rows x zero d rows)
        # claims the whole bank with start=True, and every color MM then
        # accumulates with start=False.
        acc_all = accp.tile([128, 512], f32, tag="acc")
        nc.tensor.matmul(acc_all[:], qts[0][32:64, 0:128], d_sb[32:64, 0:512],
                         start=True, stop=False, skip_group_check=True)

        def slots_of(h):
            return range(GOFF[h], GOFF[h] + GSIZES[h])

        SKEWG = 2
        whs = [None] * NGRP

        def color_mms(h):
            w = whs[h]
            for k, s in enumerate(slots_of(h)):
                g, rb = s // 2, s % 2
                clhs = col_sb[:, g * 4 : (g + 1) * 4]
                nc.tensor.matmul(acc_all[32 * rb : 32 * rb + 4, :], clhs,
                                 w[:, k * 512 : (k + 1) * 512],
                                 start=False, stop=(s >= NSLOT - 2),
                                 tile_position=(0, 32 * rb),
                                 skip_group_check=True)
            whs[h] = None

        for h in range(NGRP):
            ns = GSIZES[h]
            targ = argp.tile([128, ns * 512], f32, tag=f"arg{h % 2}")
            for k, s in enumerate(slots_of(h)):
                g, rb = s // 2, s % 2
                p = next(i for i in range(len(POFF)) if POFF[i] <= g * 128 < POFF[i] + PIECES[i])
                lhs = qts[p][:, g * 128 - POFF[p] :][:, 0:128]
                nc.tensor.matmul(targ[:, k * 512 : (k + 1) * 512], lhs,
                                 d_sb[:, rb * 512 : (rb + 1) * 512],
                                 start=True, stop=True)

            if h >= SKEWG:
                color_mms(h - SKEWG)

            w = wpool.tile([128, ns * 512], mybir.dt.float16, tag=f"w{h % 2}")
            nc.scalar.activation(w[:], targ[:], Exp)
            whs[h] = w

        for h in range(NGRP - SKEWG, NGRP):
            color_mms(h)

        out_sb = outp.tile([4, P_CORE], f32)
        nc.vector.tensor_copy(out_sb[:, 0:512], acc_all[0:4, :])
        nc.vector.tensor_copy(out_sb[:, 512:1024], acc_all[32:36, :])
        nc.sync.dma_start(out.ap(), out_sb[:])

    nc.compile()
    _module_cache["nc"] = nc
    return nc


def _run(in_maps, trace=False, **kw):
    from concourse import bass_utils

    nc = _build_module()
    return bass_utils.run_bass_kernel_spmd(
        nc, in_maps, core_ids=list(range(N_CORES)), trace=trace, **kw
    )


def _assemble(results):
    out = np.zeros((N_IMGS, 3, P_IMG), np.float32)
    for core in range(N_CORES):
        b = core // (N_CORES // N_IMGS)
        rb = core % (N_CORES // N_IMGS)
        o = results[core]["out"]                    # [4, P_CORE] f32
        rgb = o[:3] / (o[3:4] + EPS)
        out[b, :, rb * P_CORE : (rb + 1) * P_CORE] = rgb
    return out


def kernel(means, covs, colors, intrinsics, extrinsics, image_ids):
    in_maps = _host_prep(means, covs, colors, intrinsics, extrinsics, image_ids)
    res = _run(in_maps)
    return _assemble(res.results)
